# revision 3
# baseline (speedup 1.0000x reference)
"""Sliding-window causal attention (window=1024) for B=2,T=2048,H=16,D=128
on 8 trn2 NeuronCores. Shards the 32 (batch, head) pairs 4-per-core.

v3: all transposes handled on the host. q and k are passed pre-transposed
[D, T] fp16 so they load straight into the [d, t] layout the band matmuls
want; the output is stored as O^T [D, T] fp16 and transposed back on the
host. On-chip work is only: S^T = K @ Q^T per 128x128 block pair into a
double-bank PSUM tile, one wide exp on the scalar engine, PV and the
ones-matmul softmax denominators accumulated in PSUM, then a broadcast
reciprocal (reciprocal_approx_fast) normalizes O^T before the store.
"""
import math

import numpy as np

import concourse.bass as bass
import concourse.bacc as bacc
import concourse.mybir as mybir
from concourse import tile
from concourse.bass_utils import run_bass_kernel_spmd

B, T, H, D = 2, 2048, 16, 128
WINDOW = 1024
NCORES = 8
BH = B * H                  # 32 (b,h) pairs
BH_PER_CORE = BH // NCORES  # 4
NT = T // 128               # 16 seq tiles
G = 4                       # q-tiles per group (512 queries)
NG = NT // G
WB = WINDOW // 128          # window in blocks

f32 = mybir.dt.float32
f16 = mybir.dt.float16
AF = mybir.ActivationFunctionType
ALU = mybir.AluOpType


def band_blocks(g):
    """Key blocks intersecting group g's sliding band, with the trimmed
    q-tile range [t_min, t_max] each block must serve."""
    out = []
    for b in range(max(0, G * g - WB), G * g + G):
        t_min = max(G * g, b)
        t_max = min(G * g + G - 1, b + WB)
        if t_min <= t_max:
            out.append((b, t_min, t_max))
    return out


def build_nc(n_bh=BH_PER_CORE):
    nc = bacc.Bacc()
    q = nc.declare_dram_parameter("q", [n_bh, D, T], f16, isOutput=False)
    k = nc.declare_dram_parameter("k", [n_bh, D, T], f16, isOutput=False)
    v = nc.declare_dram_parameter("v", [n_bh, T, D], f16, isOutput=False)
    o = nc.declare_dram_parameter("o", [n_bh, D, T], f16, isOutput=True)

    scale = 1.0 / math.sqrt(D)

    with tile.TileContext(nc) as tc:
        with (
            tc.tile_pool(name="const", bufs=1) as constp,
            tc.tile_pool(name="io", bufs=2) as iop,
            tc.tile_pool(name="es", bufs=6) as esp,
            tc.tile_pool(name="outp", bufs=2) as outp,
            tc.tile_pool(name="ps_st", bufs=2, space="PSUM") as ps_st,
            tc.tile_pool(name="ps_pv", bufs=2, space="PSUM") as ps_pv,
            tc.tile_pool(name="ps_sum", bufs=2, space="PSUM") as ps_sum,
        ):
            def prefetch(bh):
                qt = iop.tile([128, T], f16, tag="qt", name=f"qt_{bh}")
                kt = iop.tile([128, T], f16, tag="kt", name=f"kt_{bh}")
                vb = iop.tile([128, NT, 128], f16, tag="vb", name=f"vb_{bh}")
                nc.sync.dma_start(out=kt[:], in_=k[bh])
                nc.sync.dma_start(out=qt[:], in_=q[bh])
                nc.sync.dma_start(
                    out=vb[:], in_=v[bh].rearrange("(n p) d -> p n d", p=128))
                return qt, kt, vb

            loaded = {0: prefetch(0)}

            # --- constants: ones for the denominator matmul, causal masks
            ones_f = constp.tile([128, 128], f32)
            mdiag_f = constp.tile([128, 128], f32)
            madiag_f = constp.tile([128, 128], f32)
            nc.gpsimd.memset(ones_f[:], 1.0)
            # diag mask (allowed k <= q): keep where col - p >= 0
            nc.gpsimd.affine_select(
                out=mdiag_f[:], in_=ones_f[:], compare_op=ALU.is_ge,
                fill=0.0, base=0, channel_multiplier=-1, pattern=[[1, 128]],
            )
            # anti-diag mask (allowed k > q): keep where p - col - 1 >= 0
            nc.gpsimd.affine_select(
                out=madiag_f[:], in_=ones_f[:], compare_op=ALU.is_ge,
                fill=0.0, base=-1, channel_multiplier=1, pattern=[[-1, 128]],
            )
            ones = constp.tile([128, 128], f16)
            mdiag = constp.tile([128, 128], f16)
            madiag = constp.tile([128, 128], f16)
            nc.vector.tensor_copy(ones[:], ones_f[:])
            nc.vector.tensor_copy(mdiag[:], mdiag_f[:])
            nc.vector.tensor_copy(madiag[:], madiag_f[:])

            for bh in range(n_bh):
                qt, kt, vb = loaded.pop(bh)

                for g in range(NG):
                    blocks = sorted(band_blocks(g), key=lambda x: x[1] - x[2])
                    n = len(blocks)
                    # pair widest with narrowest: first slot is always a
                    # full 512-wide block at PSUM offset 0
                    pairs = [(blocks[i], blocks[n - 1 - i])
                             for i in range(n // 2)]
                    pv = ps_pv.tile([128, 512], f32, tag="pv")
                    sm = ps_sum.tile([128, 512], f32, tag="sm")

                    def emit_pvsm(pair_idx, es, pair):
                        first = pair_idx == 0
                        last = pair_idx == len(pairs) - 1
                        for j, (b, t_min, t_max) in enumerate(pair):
                            w = (t_max - t_min + 1) * 128
                            off = (t_min - G * g) * 128
                            eo = 512 * j
                            nc.tensor.matmul(
                                pv[:, off:off + w], vb[:, b, :],
                                es[:, eo:eo + w],
                                start=first and j == 0, stop=last and j == 1)
                            nc.tensor.matmul(
                                sm[:, off:off + w], ones[:],
                                es[:, eo:eo + w],
                                start=first and j == 0, stop=last and j == 1)

                    pending = None
                    for pi, pair in enumerate(pairs):
                        stp = ps_st.tile([128, 1024], f32, tag="st")
                        es = esp.tile([128, 1024], f16, tag="es")
                        w1 = 0
                        for j, (b, t_min, t_max) in enumerate(pair):
                            w = (t_max - t_min + 1) * 128
                            nc.tensor.matmul(
                                stp[:, 512 * j:512 * j + w],
                                kt[:, 128 * b:128 * b + 128],
                                qt[:, 128 * t_min:128 * (t_max + 1)],
                                start=True, stop=True)
                            if j == 1:
                                w1 = w
                        # one exp across both blocks (the [w0, 512) gap is
                        # garbage but never read downstream)
                        nc.scalar.activation(
                            es[:, 0:512 + w1], stp[:, 0:512 + w1], AF.Exp,
                            scale=scale)
                        # causal trim masks on the band edges
                        for j, (b, t_min, t_max) in enumerate(pair):
                            w = (t_max - t_min + 1) * 128
                            eo = 512 * j
                            if b >= G * g:
                                nc.vector.tensor_mul(
                                    es[:, eo:eo + 128], es[:, eo:eo + 128],
                                    mdiag[:])
                            if b + WB <= G * g + G - 1:
                                nc.gpsimd.tensor_mul(
                                    es[:, eo + w - 128:eo + w],
                                    es[:, eo + w - 128:eo + w], madiag[:])
                        if pending is not None:
                            emit_pvsm(pending[0], pending[1], pending[2])
                        pending = (pi, es, pair)
                    emit_pvsm(pending[0], pending[1], pending[2])

                    # --- normalize pre-transpose with broadcast reciprocal;
                    # store O^T, host transposes back
                    rec = outp.tile([128, 512], f32, tag="rec")
                    nc.vector.reciprocal_approx_fast(rec[:], sm[:])
                    otn = outp.tile([128, 512], f16, tag="otn")
                    nc.vector.tensor_mul(otn[:], pv[:], rec[:])
                    nc.sync.dma_start(
                        out=o[bh, :, 512 * g:512 * (g + 1)], in_=otn[:])

                    # prefetch next bh once the first group is in flight
                    if g == 0 and bh + 1 < n_bh:
                        loaded[bh + 1] = prefetch(bh + 1)
    if not nc.is_finalized():
        nc.finalize()
    return nc


_nc = None


def _get_nc():
    global _nc
    if _nc is None:
        _nc = build_nc()
    return _nc


def make_in_maps(q, k, v):
    # [B, T, H, D] -> [B*H, T, D] fp16; q/k additionally pre-transposed
    # to [B*H, D, T] (kernel wants the [d, t] layout)
    qs = np.ascontiguousarray(
        np.asarray(q, dtype=np.float32).transpose(0, 2, 3, 1)
        .reshape(BH, D, T)).astype(np.float16)
    ks = np.ascontiguousarray(
        np.asarray(k, dtype=np.float32).transpose(0, 2, 3, 1)
        .reshape(BH, D, T)).astype(np.float16)
    vs = np.ascontiguousarray(
        np.asarray(v, dtype=np.float32).transpose(0, 2, 1, 3)
        .reshape(BH, T, D)).astype(np.float16)
    return [
        {
            "q": qs[c * BH_PER_CORE:(c + 1) * BH_PER_CORE],
            "k": ks[c * BH_PER_CORE:(c + 1) * BH_PER_CORE],
            "v": vs[c * BH_PER_CORE:(c + 1) * BH_PER_CORE],
        }
        for c in range(NCORES)
    ]


def assemble_out(results):
    # results hold O^T [n_bh, D, T] fp16 -> [BH, T, D] f32
    out = np.empty((BH, T, D), np.float32)
    for c in range(NCORES):
        ot = np.asarray(results[c]["o"], dtype=np.float32)  # [n_bh, D, T]
        out[c * BH_PER_CORE:(c + 1) * BH_PER_CORE] = ot.transpose(0, 2, 1)
    return np.ascontiguousarray(
        out.reshape(B, H, T, D).transpose(0, 2, 1, 3))


def kernel(q, k, v, window_size):
    assert int(window_size) == WINDOW
    in_maps = make_in_maps(q, k, v)
    res = run_bass_kernel_spmd(_get_nc(), in_maps, list(range(NCORES))).results
    return assemble_out(res)
